# revision 5
# baseline (speedup 1.0000x reference)
"""GCN (2-layer graph conv + log_softmax) on 8 Trainium2 NeuronCores.

Strategy: partition destination nodes (12500/core). Edges are bucketed on the
host by (dest window of 128 rows, source range of 25000 rows) and padded so
all 8 cores run one identical (SPMD) fully-unrolled program. Dense transforms
run on the tensor engine; the sparse A @ support aggregation runs as
dma_gather (COO gather of support rows) + per-128-edge-chunk indicator
matmuls accumulated in PSUM. Weights are replicated; support tables are
AllGathered between layers.
"""
import sys

sys.path.insert(0, "/opt/trn_rl_repo")

import numpy as np

import concourse.bass as bass  # noqa: F401
import concourse.mybir as mybir
import concourse.tile as tile
from concourse import bacc, bass_utils

# problem shape (hardcoded per contract)
N_NODES = 100000
N_CORES = 8
PC = N_NODES // N_CORES        # 12500 dest rows per core
IN_F = 512
HID = 256
NCLS = 40
NPAD = 64                      # class dim padded for 256B gather rows
P = 128
NWIN = (PC + P - 1) // P       # 98 windows (97*128 + 84)
WIN_SZ = [P] * (NWIN - 1) + [PC - P * (NWIN - 1)]
GW = 4                         # windows per gather group
NGRP = (NWIN + GW - 1) // GW   # 25
NRNG = 4                       # source ranges (int16 gather index limit)
RNG = N_NODES // NRNG          # 25000 rows per range table
SUB = PC // NRNG               # 3125 own rows per range
f32 = mybir.dt.float32
i16 = mybir.dt.int16
F = mybir.ActivationFunctionType
ALU = mybir.AluOpType

_cache = {}


def _prep(edge_row, edge_col, edge_val):
    """Bucket/pad edges into the uniform SPMD stream. Returns per-core arrays
    and the uniform structure."""
    er = edge_row.astype(np.int64)
    ec = edge_col.astype(np.int64)
    ev = edge_val.astype(np.float32)

    core = er // PC
    rloc = er % PC
    win = rloc // P                      # 0..97
    dstl = (rloc % P).astype(np.float32)
    # striped ranges: source s on core kc=s//PC at local row sl=s%PC belongs
    # to range sl//SUB with local table index kc*SUB + sl%SUB (each range is a
    # separate offset-0 DRAM table; dma_gather ignores AP offsets)
    rng = (ec % PC) // SUB               # 0..3
    coll = ((ec // PC) * SUB + (ec % PC) % SUB).astype(np.int16)
    grp = win // GW

    order = np.lexsort((ec, win, rng, grp, core))
    core_s = core[order]
    win_s = win[order]
    rng_s = rng[order]
    dstl_s = dstl[order]
    coll_s = coll[order]
    val_s = ev[order]

    # per-(core, win, rng) counts
    key = (core_s * NWIN + win_s) * NRNG + rng_s
    cnt = np.bincount(key, minlength=N_CORES * NWIN * NRNG).reshape(
        N_CORES, NWIN, NRNG
    )
    # uniform chunk counts: max over cores, in 128-edge chunks
    n_chunks = -(-cnt.max(axis=0) // P)            # [NWIN, NRNG]

    # bucket sequence order: for g, for r, for w in g
    seq_w, seq_r = [], []
    for g in range(NGRP):
        ws = range(g * GW, min((g + 1) * GW, NWIN))
        for r in range(NRNG):
            for w in ws:
                seq_w.append(w)
                seq_r.append(r)
    seq_w = np.array(seq_w)
    seq_r = np.array(seq_r)
    NSEQ = len(seq_w)
    nck_seq = n_chunks[seq_w, seq_r]               # chunks per bucket
    chunk_off_seq = np.concatenate([[0], np.cumsum(nck_seq)])
    NCH_TOT = int(chunk_off_seq[-1])
    SLOT_TOT = NCH_TOT * P

    seq_of = np.full((NWIN, NRNG), -1, np.int64)
    for s in range(NSEQ):
        seq_of[seq_w[s], seq_r[s]] = s
    slot_off_seq = chunk_off_seq[:-1] * P

    # call table: per (g, r): n_call chunks + offsets
    calls = []          # (g, r, n_call, chunk_off)
    for g in range(NGRP):
        ws = list(range(g * GW, min((g + 1) * GW, NWIN)))
        for r in range(NRNG):
            s0 = seq_of[ws[0], r]
            n_call = int(sum(n_chunks[w, r] for w in ws))
            calls.append((g, r, n_call, int(chunk_off_seq[s0])))

    # chunk -> window map + start flags (uniform)
    win_of_chunk = np.repeat(seq_w, nck_seq)
    start_flag = np.zeros(NCH_TOT, bool)
    first_done = set()
    for s in range(NSEQ):
        w = seq_w[s]
        if nck_seq[s] > 0 and w not in first_done:
            start_flag[chunk_off_seq[s]] = True
            first_done.add(w)
    assert len(first_done) == NWIN

    # per-core slot fill
    IDX = np.zeros((N_CORES, P, NCH_TOT * 8), np.int16)
    DST = np.zeros((N_CORES, P, NCH_TOT), np.float32)
    VAL = np.zeros((N_CORES, P, NCH_TOT), np.float32)
    core_bounds = np.searchsorted(core_s, np.arange(N_CORES + 1))
    for k in range(N_CORES):
        a, b = core_bounds[k], core_bounds[k + 1]
        sk = seq_of[win_s[a:b], rng_s[a:b]]
        cnt_seq = np.bincount(sk, minlength=NSEQ)
        runstart = np.concatenate([[0], np.cumsum(cnt_seq)])[:-1]
        pos = np.arange(b - a) - runstart[sk]
        slots = slot_off_seq[sk] + pos
        col_arr = np.zeros(SLOT_TOT, np.int16)
        dst_arr = np.full(SLOT_TOT, -1.0, np.float32)
        val_arr = np.zeros(SLOT_TOT, np.float32)
        col_arr[slots] = coll_s[a:b]
        dst_arr[slots] = dstl_s[a:b]
        val_arr[slots] = val_s[a:b]
        # wrap per call
        for (g, r, n_call, c0) in calls:
            if n_call == 0:
                continue
            sl = slice(c0 * P, (c0 + n_call) * P)
            cw = col_arr[sl].reshape(n_call * 8, 16).T          # [16, 8n]
            IDX[k, :, c0 * 8:(c0 + n_call) * 8] = np.tile(cw, (8, 1))
            DST[k, :, c0:c0 + n_call] = dst_arr[sl].reshape(n_call, P).T
            VAL[k, :, c0:c0 + n_call] = val_arr[sl].reshape(n_call, P).T

    structure = dict(
        calls=calls,
        win_of_chunk=win_of_chunk,
        start_flag=start_flag,
        NCH_TOT=NCH_TOT,
        NMAX=int(max(c[2] for c in calls)),
    )
    return structure, IDX, DST, VAL


def _build(st):
    """Build the SPMD Bass program (identical across cores)."""
    NCH_TOT = st["NCH_TOT"]
    NMAX = st["NMAX"]
    calls = st["calls"]
    win_of_chunk = st["win_of_chunk"]
    start_flag = st["start_flag"]

    nc = bacc.Bacc("TRN2", target_bir_lowering=False, debug=False,
                   num_devices=N_CORES)
    xT = nc.dram_tensor("xT", [IN_F, PC], f32, kind="ExternalInput")
    w1 = nc.dram_tensor("w1", [IN_F, HID], f32, kind="ExternalInput")
    w2p = nc.dram_tensor("w2p", [HID, NPAD], f32, kind="ExternalInput")
    b1r = nc.dram_tensor("b1r", [1, HID], f32, kind="ExternalInput")
    b2r = nc.dram_tensor("b2r", [1, NPAD], f32, kind="ExternalInput")
    iota_in = nc.dram_tensor("iota", [P, P], f32, kind="ExternalInput")
    ident_in = nc.dram_tensor("ident", [P, P], f32, kind="ExternalInput")
    IDX_in = nc.dram_tensor("IDX", [P, NCH_TOT * 8], i16, kind="ExternalInput")
    DST_in = nc.dram_tensor("DST", [P, NCH_TOT], f32, kind="ExternalInput")
    VAL_in = nc.dram_tensor("VAL", [P, NCH_TOT], f32, kind="ExternalInput")
    out_d = nc.dram_tensor("out", [PC, NCLS], f32, kind="ExternalOutput")

    xT_r = xT.ap().rearrange("(a p) n -> a p n", p=P)     # [4, 128, PC]
    w1_r = w1.ap().rearrange("(a p) n -> a p n", p=P)     # [4, 128, HID]
    w2_r = w2p.ap().rearrange("(a p) n -> a p n", p=P)    # [2, 128, NPAD]

    with tile.TileContext(nc) as tc:
        with (
            tc.tile_pool(name="const", bufs=1) as cp,
            tc.tile_pool(name="dram", bufs=1, space="DRAM") as dp,
        ):
            sup1_own = [dp.tile([SUB, HID], f32, name=f"s1o_{r}")
                        for r in range(NRNG)]
            sup1_full = [dp.tile([RNG, HID], f32, addr_space="Shared",
                                 name=f"s1f_{r}") for r in range(NRNG)]
            sup2_own = [dp.tile([SUB, NPAD], f32, name=f"s2o_{r}")
                        for r in range(NRNG)]
            sup2_full = [dp.tile([RNG, NPAD], f32, addr_space="Shared",
                                 name=f"s2f_{r}") for r in range(NRNG)]

            def split_write(dst_tiles, row0, nrows, src_tile):
                # write src_tile[0:nrows] to own rows [row0, row0+nrows),
                # split at SUB boundaries into the per-range own tiles
                p = row0
                while p < row0 + nrows:
                    r = p // SUB
                    pe = min((r + 1) * SUB, row0 + nrows)
                    nc.sync.dma_start(dst_tiles[r][p - r * SUB:pe - r * SUB, :],
                                      src_tile[p - row0:pe - row0, :])
                    p = pe

            w1t = cp.tile([P, 4, HID], f32)
            w2t = cp.tile([P, 2, NPAD], f32)
            b1t = cp.tile([1, HID], f32)
            b2t = cp.tile([1, NPAD], f32)
            iota_t = cp.tile([P, P], f32)
            ident_t = cp.tile([P, P], f32)
            ones_t = cp.tile([1, P], f32)
            for j in range(4):
                nc.sync.dma_start(w1t[:, j, :], w1_r[j])
            for j in range(2):
                nc.sync.dma_start(w2t[:, j, :], w2_r[j])
            nc.sync.dma_start(b1t[:], b1r[:])
            nc.sync.dma_start(b2t[:], b2r[:])
            nc.sync.dma_start(iota_t[:], iota_in[:])
            nc.sync.dma_start(ident_t[:], ident_in[:])
            nc.vector.memset(ones_t[:], 1.0)

            # ---------- phase A: support1_own = x_k @ w1 ----------
            with (
                tc.tile_pool(name="pa", bufs=3) as pa,
                tc.tile_pool(name="pa_ps", bufs=2, space="PSUM") as pa_ps,
            ):
                for t in range(NWIN):
                    nt = WIN_SZ[t]
                    c0 = t * P
                    xt = pa.tile([P, 4, P], f32, tag="xt")
                    for j in range(4):
                        nc.sync.dma_start(xt[:, j, :nt], xT_r[j][:, c0:c0 + nt])
                    ps = pa_ps.tile([P, HID], f32, tag="ps")
                    for j in range(4):
                        nc.tensor.matmul(
                            out=ps[:nt, :], lhsT=xt[:, j, :nt], rhs=w1t[:, j, :],
                            start=(j == 0), stop=(j == 3),
                        )
                    stt = pa.tile([P, HID], f32, tag="st")
                    nc.scalar.activation(stt[:nt, :], ps[:nt, :], F.Copy)
                    split_write(sup1_own, c0, nt, stt)

            for r in range(NRNG):
                nc.gpsimd.collective_compute(
                    "AllGather", ALU.bypass,
                    replica_groups=[list(range(N_CORES))],
                    ins=[sup1_own[r].opt()], outs=[sup1_full[r].opt()],
                )

            # ---------- phase B: layer-1 aggregation + h @ w2 ----------
            ci = 0  # global call index
            with (
                tc.tile_pool(name="gp1", bufs=2) as gp,
                tc.tile_pool(name="meta", bufs=3) as mp,
                tc.tile_pool(name="indp", bufs=4) as indp,
                tc.tile_pool(name="aggp", bufs=4, space="PSUM") as aggp,
                tc.tile_pool(name="fin", bufs=2) as fin,
                tc.tile_pool(name="finp", bufs=2, space="PSUM") as finp,
            ):
                for g in range(NGRP):
                    ws = list(range(g * GW, min((g + 1) * GW, NWIN)))
                    psums = {w: aggp.tile([P, HID], f32, tag="agg", name=f"agg_{w}")
                              for w in ws}
                    for r in range(NRNG):
                        _, _, n, c0 = calls[g * NRNG + r]
                        if n == 0:
                            continue
                        idx_t = mp.tile([P, NMAX * 8], i16, tag="idx")
                        nc.sync.dma_start(idx_t[:, :8 * n],
                                          IDX_in[:, c0 * 8:(c0 + n) * 8])
                        dst_t = mp.tile([P, NMAX], f32, tag="dst")
                        val_t = mp.tile([P, NMAX], f32, tag="val")
                        nc.sync.dma_start(dst_t[:, :n], DST_in[:, c0:c0 + n])
                        nc.sync.dma_start(val_t[:, :n], VAL_in[:, c0:c0 + n])
                        msgs = gp.tile([P, NMAX, HID], f32, tag="m1")
                        nc.gpsimd.dma_gather(
                            msgs[:, :n, :], sup1_full[r][:],
                            idx_t[:, :8 * n],
                            num_idxs=P * n, num_idxs_reg=P * n, elem_size=HID,
                            single_packet=False,
                        )
                        for j in range(n):
                            w = int(win_of_chunk[c0 + j])
                            ind = indp.tile([P, P], f32, tag="ind")
                            nc.vector.tensor_scalar(
                                ind[:], iota_t[:], dst_t[:, j:j + 1],
                                val_t[:, j:j + 1],
                                op0=ALU.is_equal, op1=ALU.mult,
                            )
                            nc.tensor.matmul(
                                out=psums[w][:], lhsT=ind[:], rhs=msgs[:, j, :],
                                start=bool(start_flag[c0 + j]), stop=False,
                            )
                        ci += 1
                    for w in ws:
                        nw = WIN_SZ[w]
                        w0 = w * P
                        nc.tensor.matmul(out=psums[w][:], lhsT=ones_t[:, :P],
                                         rhs=b1t[:], start=False, stop=True)
                        h = fin.tile([P, HID], f32, tag="h")
                        nc.scalar.activation(h[:nw, :], psums[w][:nw, :], F.Relu)
                        hT = fin.tile([P, 2, P], f32, tag="hT")
                        for j in range(2):
                            pt = finp.tile([P, P], f32, tag="pt")
                            nc.tensor.transpose(
                                out=pt[:, :nw], in_=h[:nw, j * P:(j + 1) * P],
                                identity=ident_t[:nw, :nw],
                            )
                            nc.vector.tensor_copy(hT[:, j, :nw], pt[:, :nw])
                        ps2 = finp.tile([P, NPAD], f32, tag="ps2")
                        for j in range(2):
                            nc.tensor.matmul(
                                out=ps2[:nw, :], lhsT=hT[:, j, :nw],
                                rhs=w2t[:, j, :],
                                start=(j == 0), stop=(j == 1),
                            )
                        s2 = fin.tile([P, NPAD], f32, tag="s2")
                        nc.vector.tensor_copy(s2[:nw, :], ps2[:nw, :])
                        split_write(sup2_own, w0, nw, s2)

            for r in range(NRNG):
                nc.gpsimd.collective_compute(
                    "AllGather", ALU.bypass,
                    replica_groups=[list(range(N_CORES))],
                    ins=[sup2_own[r].opt()], outs=[sup2_full[r].opt()],
                )

            # ---------- phase C: layer-2 aggregation + log_softmax ----------
            with (
                tc.tile_pool(name="gp2", bufs=2) as gp2,
                tc.tile_pool(name="meta2", bufs=3) as mp2,
                tc.tile_pool(name="indp2", bufs=4) as indp2,
                tc.tile_pool(name="aggp2", bufs=4, space="PSUM") as aggp2,
                tc.tile_pool(name="fin2", bufs=3) as fin2,
            ):
                for g in range(NGRP):
                    ws = list(range(g * GW, min((g + 1) * GW, NWIN)))
                    psums = {w: aggp2.tile([P, NPAD], f32, tag="agg2",
                                            name=f"agg2_{w}")
                             for w in ws}
                    for r in range(NRNG):
                        _, _, n, c0 = calls[g * NRNG + r]
                        if n == 0:
                            continue
                        idx_t = mp2.tile([P, NMAX * 8], i16, tag="idx2")
                        nc.sync.dma_start(idx_t[:, :8 * n],
                                          IDX_in[:, c0 * 8:(c0 + n) * 8])
                        dst_t = mp2.tile([P, NMAX], f32, tag="dst2")
                        val_t = mp2.tile([P, NMAX], f32, tag="val2")
                        nc.sync.dma_start(dst_t[:, :n], DST_in[:, c0:c0 + n])
                        nc.sync.dma_start(val_t[:, :n], VAL_in[:, c0:c0 + n])
                        msgs = gp2.tile([P, NMAX, NPAD], f32, tag="m2")
                        nc.gpsimd.dma_gather(
                            msgs[:, :n, :], sup2_full[r][:],
                            idx_t[:, :8 * n],
                            num_idxs=P * n, num_idxs_reg=P * n, elem_size=NPAD,
                            single_packet=False,
                        )
                        for j in range(n):
                            w = int(win_of_chunk[c0 + j])
                            ind = indp2.tile([P, P], f32, tag="ind2")
                            nc.vector.tensor_scalar(
                                ind[:], iota_t[:], dst_t[:, j:j + 1],
                                val_t[:, j:j + 1],
                                op0=ALU.is_equal, op1=ALU.mult,
                            )
                            nc.tensor.matmul(
                                out=psums[w][:], lhsT=ind[:], rhs=msgs[:, j, :],
                                start=bool(start_flag[c0 + j]), stop=False,
                            )
                    for w in ws:
                        nw = WIN_SZ[w]
                        w0 = w * P
                        nc.tensor.matmul(out=psums[w][:], lhsT=ones_t[:, :P],
                                         rhs=b2t[:], start=False, stop=True)
                        mx = fin2.tile([P, 1], f32, tag="mx")
                        nc.vector.reduce_max(mx[:nw], psums[w][:nw, :NCLS],
                                             axis=mybir.AxisListType.X)
                        tt = fin2.tile([P, NCLS], f32, tag="tt")
                        nc.vector.tensor_scalar(tt[:nw, :], psums[w][:nw, :NCLS],
                                                mx[:nw], None, op0=ALU.subtract)
                        ex = fin2.tile([P, NCLS], f32, tag="ex")
                        sm = fin2.tile([P, 1], f32, tag="sm")
                        nc.scalar.activation(ex[:nw, :], tt[:nw, :], F.Exp,
                                             accum_out=sm[:nw])
                        lg = fin2.tile([P, 1], f32, tag="lg")
                        nc.scalar.activation(lg[:nw], sm[:nw], F.Ln)
                        oo = fin2.tile([P, NCLS], f32, tag="oo")
                        nc.vector.tensor_scalar(oo[:nw, :], tt[:nw, :],
                                                lg[:nw], None, op0=ALU.subtract)
                        nc.sync.dma_start(out_d[w0:w0 + nw, :], oo[:nw, :])

    nc.compile()
    return nc


def kernel(x, w1, b1, w2, b2, edge_val, edge_row, edge_col):
    x = np.asarray(x, np.float32)
    w1 = np.asarray(w1, np.float32)
    b1 = np.asarray(b1, np.float32)
    w2 = np.asarray(w2, np.float32)
    b2 = np.asarray(b2, np.float32)
    edge_val = np.asarray(edge_val, np.float32)
    edge_row = np.asarray(edge_row)
    edge_col = np.asarray(edge_col)

    st, IDX, DST, VAL = _prep(edge_row, edge_col, edge_val)

    key = (st["NCH_TOT"], st["NMAX"])
    if key not in _cache:
        _cache[key] = _build(st)
    nc = _cache[key]

    w2p = np.zeros((HID, NPAD), np.float32)
    w2p[:, :NCLS] = w2
    b2r = np.zeros((1, NPAD), np.float32)
    b2r[0, :NCLS] = b2
    iota_np = np.tile(np.arange(P, dtype=np.float32), (P, 1))
    ident_np = np.eye(P, dtype=np.float32)

    in_maps = []
    for k in range(N_CORES):
        xk = np.ascontiguousarray(x[k * PC:(k + 1) * PC].T)
        in_maps.append({
            "xT": xk,
            "w1": w1,
            "w2p": w2p,
            "b1r": b1.reshape(1, HID),
            "b2r": b2r,
            "iota": iota_np,
            "ident": ident_np,
            "IDX": IDX[k],
            "DST": DST[k],
            "VAL": VAL[k],
        })

    global _last_in_maps
    _last_in_maps = in_maps

    res = bass_utils.run_bass_kernel_spmd(
        nc, in_maps, core_ids=list(range(N_CORES)),
    )
    out = np.concatenate([res.results[k]["out"] for k in range(N_CORES)],
                         axis=0)
    return out.astype(np.float32)


_last_in_maps = None


def _timeit():
    """Rerun the cached program with tracing; returns max-core exec ns."""
    if _last_in_maps is None or not _cache:
        return None
    nc = next(iter(_cache.values()))
    res = bass_utils.run_bass_kernel_spmd(
        nc, _last_in_maps, core_ids=list(range(N_CORES)), trace=True,
    )
    return res.exec_time_ns
